# revision 13
# baseline (speedup 1.0000x reference)
"""Causal self-attention with LoRA q/k/v projections on 8 TRN2 NeuronCores.

Problem: B=4, S=2048, H=1024, NH=16, HD=64, LoRA r=8 alpha=16 (scaling 2.0),
causal mask; attention_mask is all-zeros by construction (ignored).

Sharding (zero collectives): core c handles batch b=c//2 and head-group
hg=c%2 (8 heads = 512 projection columns). The host folds LoRA into the base
weights (W_eff = W + 2*A@B in float64), folds the 1/sqrt(64) attention scale
into Wq/bq, and ships x PRE-TRANSPOSED (xT [H, S]) plus W_eff in bf16; all
matmuls run bf16 (PSUM fp32).

Device per core (PE instructions pinned to emission order via nosync deps):
  phase A: v = x @ Wv + bv into per-head layout vp[tk, 8, 64] (no ones col).
  phase B: qT/kT = (x @ W' + b')^T in [j, t] layout, two 64-row heads per
           128-partition j-tile. j-tile 0 runs up front; j-tile p+1 is
           interleaved into pair p's attention blocks.
  phase C: per head-pair p, per tq-chunk-PAIR (c0,c1), j-outer over causal
           tk tiles: sT for both heads of the pair lands in ONE 2-bank PSUM
           tile [128, 1024] (s0 at 0:512, s1 at 512:1024) via row-packed
           K=64 matmul pairs; ONE merged exp ACTIVATE covers both heads
           (two calls only for off>0 diagonal blocks); diagonal masked by a
           single 3D-AP 0/1 lower-tri multiply covering both heads.
           AV is COL-PACKED: the two heads' [K=128, M=64] matmuls run
           concurrently in col groups 0-1 / 2-3 (tile_position (0,0)/(0,64))
           into one PSUM bank [128, 512] (head0 rows 0:64, head1 64:128).
           Softmax denominators: pt blocks are accumulated tk-tile-wise into
           ptsum [128, 1024] bf16 on DVE; after each chunk's j loop, two
           M=1 ones-stationary matmuls (col groups 0 / 2) produce den rows.
Host epilogue: divide av by den, transpose per head, scatter into
[B, S, 1024] float32.

Note: walrus in this container accepts at most ONE sync-wait per
instruction; _split_sync_waits hoists Tile's aggregated drain waits onto
NoOps - without it nothing compiles.
"""

import math

import numpy as np
from contextlib import ExitStack

import concourse.bass as bass
import concourse.tile as tile
from concourse import mybir
from concourse.bass_utils import run_bass_kernel_spmd

B, S, H = 4, 2048, 1024
NH, HD = 16, 64
LORA_SCALING = 2.0          # alpha/r = 16/8
N_CORES = 8
HPC = NH // 2               # heads per core
JW = HPC * HD               # 512 projection cols per core
TT = S // 128               # 16 t tiles
IT = H // 128               # 8 contraction tiles
JT = JW // 128              # 4 j tiles per core (= head pairs)
CH = S // 512               # 4 tq chunks
F32 = mybir.dt.float32
BF16 = mybir.dt.bfloat16


def _split_sync_waits(nc, max_waits=1):
    """walrus in this container allows ONE sync-wait per instruction; hoist
    excess waits (Tile's end drain aggregates many) onto preceding NoOps."""
    for fn in nc.m.functions:
        for bb in fn.blocks:
            insts = bb.instructions
            i = 0
            while i < len(insts):
                ins = insts[i]
                si = ins.sync_info
                ow = list(si.on_wait) if si is not None else []
                if len(ow) > max_waits:
                    keep = ow[-max_waits:]
                    excess = ow[:-max_waits]
                    for ci in range(0, len(excess), max_waits):
                        nop = mybir.InstNoOp(
                            name=f"{ins.name}-wsplit{ci}",
                            engine=ins.engine,
                            ins=[],
                            outs=[],
                            sync_info=mybir.SyncInfo(
                                on_wait=excess[ci : ci + max_waits], on_update=[]
                            ),
                        )
                        insts.insert(i, nop)
                        i += 1
                    ins.sync_info.on_wait = keep
                i += 1


def _build_program():
    nc = bass.Bass(
        "TRN2", target_bir_lowering=False, debug=False, num_devices=N_CORES
    )
    xT_ap = nc.dram_tensor("xT", [H, S], BF16, kind="ExternalInput").ap()
    wq_ap = nc.dram_tensor("wq", [H, JW], BF16, kind="ExternalInput").ap()
    wk_ap = nc.dram_tensor("wk", [H, JW], BF16, kind="ExternalInput").ap()
    wv_ap = nc.dram_tensor("wv", [H, JW], BF16, kind="ExternalInput").ap()
    bq_ap = nc.dram_tensor("bq", [128, JT], F32, kind="ExternalInput").ap()
    bk_ap = nc.dram_tensor("bk", [128, JT], F32, kind="ExternalInput").ap()
    bv_ap = nc.dram_tensor("bv", [1, JW], F32, kind="ExternalInput").ap()
    tri_ap = nc.dram_tensor("tri2", [128, 2, 128], BF16, kind="ExternalInput").ap()
    oav_ap = nc.dram_tensor("out_av", [JT, 128, S], BF16, kind="ExternalOutput").ap()
    # den rows: [hh, p*CH*512] (hh = head parity within pair)
    odn_ap = nc.dram_tensor("out_den", [2, JT * S], F32, kind="ExternalOutput").ap()

    ACT_EXP = mybir.ActivationFunctionType.Exp

    from concourse.tile import add_dep_helper

    with ExitStack() as ctx:
        tc = ctx.enter_context(tile.TileContext(nc))
        # PSUM budget (8 banks): sc 2x[128,1024]f32 = 4, av 2x[128,512] = 2,
        # pq 1, den 1.
        ps_sc = ctx.enter_context(tc.tile_pool(name="ps_sc", bufs=2, space="PSUM"))
        ps_av = ctx.enter_context(tc.tile_pool(name="ps_av", bufs=2, space="PSUM"))
        ps_pq = ctx.enter_context(tc.tile_pool(name="ps_pq", bufs=1, space="PSUM"))
        ps_dn = ctx.enter_context(tc.tile_pool(name="ps_dn", bufs=1, space="PSUM"))
        consts = ctx.enter_context(tc.tile_pool(name="consts", bufs=1))
        vp_pool = ctx.enter_context(tc.tile_pool(name="vp", bufs=TT))
        qkt_pool = ctx.enter_context(tc.tile_pool(name="qkt", bufs=1))
        pt_pool = ctx.enter_context(tc.tile_pool(name="pt", bufs=4))
        pts_pool = ctx.enter_context(tc.tile_pool(name="pts", bufs=3))
        avs_pool = ctx.enter_context(tc.tile_pool(name="avs", bufs=3))
        w_pool = ctx.enter_context(tc.tile_pool(name="w", bufs=1))
        xT_pool = ctx.enter_context(tc.tile_pool(name="xT", bufs=1))

        pe_chain = [None]

        def _pe(inst):
            if pe_chain[0] is not None:
                add_dep_helper(inst.ins, pe_chain[0].ins, sync=False, reason="pe order")
            pe_chain[0] = inst
            return inst

        # ---- constants (scalar queue: tiny consts, then weights; sync +
        # gpsimd carry xT halves; vector does memsets so no queue clogs) ----
        ones1 = consts.tile([1, 128], F32)
        nc.vector.memset(ones1[:], 1.0)
        onesb = consts.tile([128, 1], BF16)
        nc.vector.memset(onesb[:], 1.0)
        # prime the exp table set while DMAs stream (one-time ~2.7us)
        dumt = consts.tile([1, 128], F32)
        nc.scalar.activation(dumt[:], ones1[:], ACT_EXP)
        tri2 = consts.tile([128, 2, 128], BF16)  # 1 where tq>=tk else 0, both heads
        nc.scalar.dma_start(tri2[:], tri_ap[:])
        bvrow = consts.tile([1, JW], F32)
        nc.scalar.dma_start(bvrow[:], bv_ap[:])
        bq_t = consts.tile([128, JT], F32)
        nc.scalar.dma_start(bq_t[:], bq_ap[:])
        bk_t = consts.tile([128, JT], F32)
        nc.scalar.dma_start(bk_t[:], bk_ap[:])
        # den staging: partition 0 = even head of pair, partition 64 = odd
        den_all = consts.tile([65, JT * S], F32)

        # ---- weight + x DMAs (issued up front; deps gate consumers) ----
        xT = xT_pool.tile([128, IT, S], BF16)
        for half in range(2):
            for i in range(IT):
                eng = nc.sync if i % 2 == 0 else nc.gpsimd
                eng.dma_start(
                    xT[:, i, half * 1024 : (half + 1) * 1024],
                    xT_ap[i * 128 : (i + 1) * 128, half * 1024 : (half + 1) * 1024],
                )

        wv_tiles = []
        for i in range(IT):
            wv = w_pool.tile([128, JW], BF16, name=f"wv_{i}")
            nc.scalar.dma_start(wv[:], wv_ap[i * 128 : (i + 1) * 128, :])
            wv_tiles.append(wv)
        w_tiles = {}
        for key, w_ap in (("q", wq_ap), ("k", wk_ap)):
            for i in range(IT):
                w = w_pool.tile([128, JW], BF16, name=f"w{key}_{i}")
                nc.scalar.dma_start(w[:], w_ap[i * 128 : (i + 1) * 128, :])
                w_tiles[(key, i)] = w

        # v bias broadcast to all 128 partitions via ones-matmul
        bvb = consts.tile([128, JW], F32)
        bvb_ps = ps_pq.tile([128, 512], F32, tag="pq")
        _pe(nc.tensor.matmul(bvb_ps[:], ones1[:], bvrow[:], start=True, stop=True))
        nc.vector.tensor_copy(bvb[:], bvb_ps[:])

        qT = qkt_pool.tile([128, JT, S], BF16)
        kT = qkt_pool.tile([128, JT, S], BF16)

        # ---- phase A: v projection into vp[tk, head, 64] (per t tile) ----
        vp_tiles = {}

        def emit_pv(t):
            pv = ps_pq.tile([128, 512], F32, tag="pq", name=f"pv_{t}")
            for i in range(IT):
                _pe(nc.tensor.matmul(
                    pv[:],
                    xT[:, i, t * 128 : (t + 1) * 128],
                    wv_tiles[i][:],
                    start=(i == 0),
                    stop=(i == IT - 1),
                ))
            vp = vp_pool.tile([128, HPC, HD], BF16)
            nc.vector.tensor_add(
                vp[:],
                pv[:].rearrange("p (h d) -> p h d", h=HPC),
                bvb[:].rearrange("p (h d) -> p h d", h=HPC),
            )
            vp_tiles[t] = vp

        # ---- phase B: qT/kT projection chains ----
        def emit_proj_chain(key, j, c):
            b_t, dstT = (bq_t, qT) if key == "q" else (bk_t, kT)
            pq = ps_pq.tile([128, 512], F32, tag="pq", name=f"pq_{key}_{j}_{c}")
            for i in range(IT):
                _pe(nc.tensor.matmul(
                    pq[:],
                    w_tiles[(key, i)][:, j * 128 : (j + 1) * 128],
                    xT[:, i, c * 512 : (c + 1) * 512],
                    start=(i == 0),
                    stop=(i == IT - 1),
                ))
            nc.vector.tensor_scalar_add(
                dstT[:, j, c * 512 : (c + 1) * 512], pq[:], b_t[:, j : j + 1]
            )

        # work queue of projection/pv units, consumed just-in-time between
        # attention blocks (all emitted PE work stays back-to-back)
        units = []
        for c in range(1, CH):
            units += [("pv", t) for t in range(4 * c, 4 * c + 4)]
            units += [("q", 0, c), ("k", 0, c)]
        for p in range(1, JT):
            for c in range(CH):
                units += [("q", p, c), ("k", p, c)]
        unit_pos = [0]

        def consume_unit():
            u = units[unit_pos[0]]
            unit_pos[0] += 1
            if u[0] == "pv":
                emit_pv(u[1])
            else:
                emit_proj_chain(u[0], u[1], u[2])

        def drain_units(upto):
            while unit_pos[0] < upto:
                consume_unit()

        def prereq(p, c):
            if p == 0:
                return 6 * c
            return 18 + 8 * (p - 1) + 2 * (c + 1)

        # prologue: minimum for attention (p0, c0)
        for t in range(4):
            emit_pv(t)
        emit_proj_chain("q", 0, 0)
        emit_proj_chain("k", 0, 0)

        # ---- phase C: attention blocks, c-outer j-inner ----
        av_tiles = {}
        dn_tiles = {}
        pts_tiles = {}

        def emit_scores(p, c, j, off):
            N = 512 - off
            tq0 = c * 512 + off
            sc = ps_sc.tile([128, 1024], F32, tag="sc", name=f"sc_{p}_{c}_{j}")
            _pe(nc.tensor.matmul(
                sc[:, 0:N],
                kT[0:64, p, j * 128 : (j + 1) * 128],
                qT[0:64, p, tq0 : tq0 + N],
                start=True,
                stop=True,
                tile_position=(0, 0),
            ))
            _pe(nc.tensor.matmul(
                sc[:, 512 : 512 + N],
                kT[64:128, p, j * 128 : (j + 1) * 128],
                qT[64:128, p, tq0 : tq0 + N],
                start=True,
                stop=True,
                tile_position=(64, 0),
            ))
            return sc

        def emit_tail(p, c, j, off, sc):
            N = 512 - off
            jmax = 4 * c + 3
            if (p, c) not in av_tiles:
                av_tiles[(p, c)] = ps_av.tile(
                    [128, 512], F32, tag="av", name=f"av_{p}_{c}"
                )
                dn_tiles[(p, c)] = ps_dn.tile(
                    [65, 512], F32, tag="dn", name=f"dn_{p}_{c}"
                )
                pts_tiles[(p, c)] = pts_pool.tile(
                    [128, 1024], BF16, tag="pts", name=f"pts_{p}_{c}"
                )
            av = av_tiles[(p, c)]
            dn = dn_tiles[(p, c)]
            pts = pts_tiles[(p, c)]
            pt = pt_pool.tile([128, 1024], BF16, tag="pt", name=f"pt_{p}_{c}_{j}")
            # single exp covers both heads; [N:512) is stale-but-bounded data
            nc.scalar.activation(pt[:, 0 : 512 + N], sc[:, 0 : 512 + N], ACT_EXP)
            if j >= 4 * c:
                nc.vector.tensor_mul(
                    pt[:, 0:1024].rearrange("p (g q) -> p g q", g=2)[:, :, 0:128],
                    pt[:, 0:1024].rearrange("p (g q) -> p g q", g=2)[:, :, 0:128],
                    tri2[:],
                )
            _pe(nc.tensor.matmul(
                av[0:64, off : off + N],
                vp_tiles[j][:, 2 * p, :],
                pt[:, 0:N],
                start=(j == 0),
                stop=(j == jmax),
                skip_group_check=True,
                tile_position=(0, 0),
            ))
            _pe(nc.tensor.matmul(
                av[64:128, off : off + N],
                vp_tiles[j][:, 2 * p + 1, :],
                pt[:, 512 : 512 + N],
                start=(j == 0),
                stop=(j == jmax),
                skip_group_check=True,
                tile_position=(0, 64),
            ))
            if j == 0:
                nc.gpsimd.tensor_copy(pts[:, 0:1024], pt[:, 0:1024])
            else:
                psrc = pt[:, 0:1024].rearrange("p (g q) -> p g q", g=2)[:, :, 0:N]
                pdst = pts[:, 0:1024].rearrange("p (g q) -> p g q", g=2)[
                    :, :, off : off + N
                ]
                nc.gpsimd.tensor_add(pdst, pdst, psrc)
            if j == jmax:
                _pe(nc.tensor.matmul(
                    dn[0:1, :],
                    onesb[:],
                    pts[:, 0:512],
                    start=True,
                    stop=True,
                    skip_group_check=True,
                    tile_position=(0, 0),
                ))
                _pe(nc.tensor.matmul(
                    dn[64:65, :],
                    onesb[:],
                    pts[:, 512:1024],
                    start=True,
                    stop=True,
                    skip_group_check=True,
                    tile_position=(0, 64),
                ))
                o = avs_pool.tile([128, 512], BF16, tag="o", name=f"o_{p}_{c}")
                nc.vector.tensor_copy(o[:], av[:])
                nc.sync.dma_start(
                    oav_ap[p, :, c * 512 : (c + 1) * 512], o[:]
                )
                d0 = p * S + c * 512
                nc.vector.tensor_copy(
                    den_all[0:1, d0 : d0 + 512], dn[0:1, :]
                )
                nc.vector.tensor_copy(
                    den_all[64:65, d0 : d0 + 512], dn[64:65, :]
                )
                del av_tiles[(p, c)]
                del dn_tiles[(p, c)]
                del pts_tiles[(p, c)]

        pending = []
        since_unit = [0]
        for p in range(JT):
            for c in range(CH):
                drain_units(prereq(p, c))
                for j in range(4 * c + 4):
                    off = 0 if j < 4 * c else 128 * (j - 4 * c)
                    sc = emit_scores(p, c, j, off)
                    pending.append((p, c, j, off, sc))
                    if len(pending) > 2:
                        emit_tail(*pending.pop(0))
                    since_unit[0] += 1
                    if since_unit[0] >= 2 and unit_pos[0] < len(units):
                        since_unit[0] = 0
                        consume_unit()
        while pending:
            emit_tail(*pending.pop(0))
        drain_units(len(units))
        nc.sync.dma_start(odn_ap[0:1, :], den_all[0:1, :])
        nc.sync.dma_start(odn_ap[1:2, :], den_all[64:65, :])

    _split_sync_waits(nc)
    return nc


_NC_CACHE = {}


def _get_program():
    if "nc" not in _NC_CACHE:
        _NC_CACHE["nc"] = _build_program()
    return _NC_CACHE["nc"]


def _host_prep(inputs):
    scale = 1.0 / math.sqrt(HD)
    import ml_dtypes

    tri = (
        np.arange(128)[None, :] >= np.arange(128)[:, None]
    ).astype(np.float32)
    tri2 = np.ascontiguousarray(
        np.broadcast_to(tri[:, None, :], (128, 2, 128))
    ).astype(ml_dtypes.bfloat16)
    w_eff = {}
    for name in ("q", "k", "v"):
        W = np.asarray(inputs[f"W{name}"], np.float64)
        A = np.asarray(inputs[f"A{name}"], np.float64)
        Bm = np.asarray(inputs[f"B{name}"], np.float64)
        w_eff[name] = W + LORA_SCALING * (A @ Bm)
    xT_b = []
    for b in range(B):
        xb = np.asarray(inputs["hidden_states"], np.float32)[b]
        xT_b.append(np.ascontiguousarray(xb.T).astype(ml_dtypes.bfloat16))
    in_maps = []
    for c in range(N_CORES):
        b, hg = c // 2, c % 2
        sl = slice(hg * JW, (hg + 1) * JW)
        bq = np.asarray(inputs["bq"], np.float64)[sl] * scale
        bk = np.asarray(inputs["bk"], np.float64)[sl]
        bv = np.asarray(inputs["bv"], np.float64)[sl]
        in_maps.append(
            {
                "xT": xT_b[b],
                "wq": np.ascontiguousarray(
                    (w_eff["q"][:, sl] * scale)
                ).astype(ml_dtypes.bfloat16),
                "wk": np.ascontiguousarray(w_eff["k"][:, sl]).astype(ml_dtypes.bfloat16),
                "wv": np.ascontiguousarray(w_eff["v"][:, sl]).astype(ml_dtypes.bfloat16),
                "bq": np.ascontiguousarray(
                    bq.astype(np.float32).reshape(JT, 128).T
                ),
                "bk": np.ascontiguousarray(
                    bk.astype(np.float32).reshape(JT, 128).T
                ),
                "bv": bv.astype(np.float32).reshape(1, JW),
                "tri2": tri2,
            }
        )
    return in_maps


def _host_finish(results):
    out = np.empty((B, S, NH * HD), np.float32)
    for c in range(N_CORES):
        b, hg = c // 2, c % 2
        av = results[c]["out_av"].astype(np.float32).reshape(JT, 2, HD, S)
        # out_den [2, JT*S]: row hh, col p*S + tq
        den = results[c]["out_den"].reshape(2, JT, 1, S).transpose(1, 0, 2, 3)
        heads = av / den                      # [p, hh, d, tq]
        heads = heads.transpose(3, 0, 1, 2).reshape(S, JW)
        out[b, :, hg * JW : (hg + 1) * JW] = heads
    return out


def kernel(**inputs) -> np.ndarray:
    in_maps = _host_prep(inputs)
    nc = _get_program()
    res = run_bass_kernel_spmd(nc, in_maps, list(range(N_CORES)))
    return _host_finish(res.results)


if __name__ == "__main__":
    import reference

    inputs = {k: np.asarray(v) for k, v in reference.setup_inputs().items()}
    expected = np.asarray(reference.reference(**inputs))
    actual = kernel(**inputs)
    err = np.abs(actual - expected)
    print("max abs err:", err.max())
    print("scale-relative:", err.max() / np.abs(expected).max())


# revision 17
# speedup vs baseline: 1.5825x; 1.5825x over previous
"""Causal self-attention with LoRA q/k/v projections on 8 TRN2 NeuronCores.

Problem: B=4, S=2048, H=1024, NH=16, HD=64, LoRA r=8 alpha=16 (scaling 2.0),
causal mask; attention_mask is all-zeros by construction (ignored).

Sharding (zero collectives): core c handles batch b=c//2 and head-group
hg=c%2 (8 heads = 512 projection columns). The host folds LoRA into the base
weights (W_eff = W + 2*A@B in float64), folds the 1/sqrt(64) attention scale
into Wq/bq, and ships x PRE-TRANSPOSED (xT [H, S]) plus W_eff in bf16; all
matmuls run bf16 (PSUM fp32).

Device per core (PE instructions pinned to emission order via nosync deps):
  phase A: v = x @ Wv + bv into per-head layout vp[tk, 8, 64] (no ones col).
  phase B: qT/kT = (x @ W' + b')^T in [j, t] layout, two 64-row heads per
           128-partition j-tile. j-tile 0 runs up front; j-tile p+1 is
           interleaved into pair p's attention blocks.
  phase C: per head-pair p, per tq-chunk-PAIR (c0,c1), j-outer over causal
           tk tiles: sT for both heads of the pair lands in ONE 2-bank PSUM
           tile [128, 1024] (s0 at 0:512, s1 at 512:1024) via row-packed
           K=64 matmul pairs; ONE merged exp ACTIVATE covers both heads
           (two calls only for off>0 diagonal blocks); diagonal masked by a
           single 3D-AP 0/1 lower-tri multiply covering both heads.
           AV is COL-PACKED: the two heads' [K=128, M=64] matmuls run
           concurrently in col groups 0-1 / 2-3 (tile_position (0,0)/(0,64))
           into one PSUM bank [128, 512] (head0 rows 0:64, head1 64:128).
           Softmax denominators: pt blocks are accumulated tk-tile-wise into
           ptsum [128, 1024] bf16 on DVE; after each chunk's j loop, two
           M=1 ones-stationary matmuls (col groups 0 / 2) produce den rows.
Host epilogue: divide av by den, transpose per head, scatter into
[B, S, 1024] float32.

Note: walrus in this container accepts at most ONE sync-wait per
instruction; _split_sync_waits hoists Tile's aggregated drain waits onto
NoOps - without it nothing compiles.
"""

import math

import numpy as np
from contextlib import ExitStack

import concourse.bass as bass
import concourse.tile as tile
from concourse import mybir
from concourse.bass_utils import run_bass_kernel_spmd

B, S, H = 4, 2048, 1024
NH, HD = 16, 64
LORA_SCALING = 2.0          # alpha/r = 16/8
N_CORES = 8
HPC = NH // 2               # heads per core
JW = HPC * HD               # 512 projection cols per core
TT = S // 128               # 16 t tiles
IT = H // 128               # 8 contraction tiles
JT = JW // 128              # 4 j tiles per core (= head pairs)
CH = S // 512               # 4 tq chunks
F32 = mybir.dt.float32
BF16 = mybir.dt.bfloat16


def _split_sync_waits(nc, max_waits=1):
    """walrus in this container allows ONE sync-wait per instruction; hoist
    excess waits (Tile's end drain aggregates many) onto preceding NoOps."""
    for fn in nc.m.functions:
        for bb in fn.blocks:
            insts = bb.instructions
            i = 0
            while i < len(insts):
                ins = insts[i]
                si = ins.sync_info
                ow = list(si.on_wait) if si is not None else []
                if len(ow) > max_waits:
                    keep = ow[-max_waits:]
                    excess = ow[:-max_waits]
                    for ci in range(0, len(excess), max_waits):
                        nop = mybir.InstNoOp(
                            name=f"{ins.name}-wsplit{ci}",
                            engine=ins.engine,
                            ins=[],
                            outs=[],
                            sync_info=mybir.SyncInfo(
                                on_wait=excess[ci : ci + max_waits], on_update=[]
                            ),
                        )
                        insts.insert(i, nop)
                        i += 1
                    ins.sync_info.on_wait = keep
                i += 1


def _build_program():
    nc = bass.Bass(
        "TRN2", target_bir_lowering=False, debug=False, num_devices=N_CORES
    )
    xT_ap = nc.dram_tensor("xT", [H, S], BF16, kind="ExternalInput").ap()
    wq_ap = nc.dram_tensor("wq", [H, JW], BF16, kind="ExternalInput").ap()
    wk_ap = nc.dram_tensor("wk", [H, JW], BF16, kind="ExternalInput").ap()
    wv_ap = nc.dram_tensor("wv", [H, JW], BF16, kind="ExternalInput").ap()
    bq_ap = nc.dram_tensor("bq", [128, JT], F32, kind="ExternalInput").ap()
    bk_ap = nc.dram_tensor("bk", [128, JT], F32, kind="ExternalInput").ap()
    bv_ap = nc.dram_tensor("bv", [1, JW], BF16, kind="ExternalInput").ap()
    tri_ap = nc.dram_tensor("tri2", [128, 2, 128], BF16, kind="ExternalInput").ap()
    oav_ap = nc.dram_tensor("out_av", [JT, 128, S], BF16, kind="ExternalOutput").ap()
    # den rows: [hh, p*CH*512] (hh = head parity within pair)
    odn_ap = nc.dram_tensor("out_den", [2, JT * S], F32, kind="ExternalOutput").ap()

    ACT_EXP = mybir.ActivationFunctionType.Exp

    from concourse.tile import add_dep_helper

    with ExitStack() as ctx:
        tc = ctx.enter_context(tile.TileContext(nc))
        # PSUM budget (8 banks): sc 2x[128,1024]f32 = 4, av 2x[128,512] = 2,
        # pq 1, den 1.
        ps_sc = ctx.enter_context(tc.tile_pool(name="ps_sc", bufs=2, space="PSUM"))
        ps_av = ctx.enter_context(tc.tile_pool(name="ps_av", bufs=2, space="PSUM"))
        ps_pq = ctx.enter_context(tc.tile_pool(name="ps_pq", bufs=1, space="PSUM"))
        ps_dn = ctx.enter_context(tc.tile_pool(name="ps_dn", bufs=1, space="PSUM"))
        consts = ctx.enter_context(tc.tile_pool(name="consts", bufs=1))
        vp_pool = ctx.enter_context(tc.tile_pool(name="vp", bufs=TT))
        qkt_pool = ctx.enter_context(tc.tile_pool(name="qkt", bufs=1))
        pt_pool = ctx.enter_context(tc.tile_pool(name="pt", bufs=4))
        pts_pool = ctx.enter_context(tc.tile_pool(name="pts", bufs=3))
        avs_pool = ctx.enter_context(tc.tile_pool(name="avs", bufs=3))
        w_pool = ctx.enter_context(tc.tile_pool(name="w", bufs=1))
        xT_pool = ctx.enter_context(tc.tile_pool(name="xT", bufs=1))

        pe_chain = [None]

        def _pe(inst):
            if pe_chain[0] is not None:
                add_dep_helper(inst.ins, pe_chain[0].ins, sync=False, reason="pe order")
            pe_chain[0] = inst
            return inst

        # ---- constants (scalar queue: tiny consts, then weights; sync +
        # gpsimd carry xT halves; vector does memsets so no queue clogs) ----
        ones1 = consts.tile([1, 128], BF16)
        nc.vector.memset(ones1[:], 1.0)
        onesb = consts.tile([128, 1], BF16)
        nc.vector.memset(onesb[:], 1.0)
        # prime the exp table set while DMAs stream (one-time ~2.7us)
        dumt = consts.tile([1, 128], F32)
        nc.scalar.activation(dumt[:], ones1[:], ACT_EXP)
        tri2 = consts.tile([128, 2, 128], BF16)  # 1 where tq>=tk else 0, both heads
        nc.scalar.dma_start(tri2[:], tri_ap[:])
        bvrow = consts.tile([1, JW], BF16)
        nc.scalar.dma_start(bvrow[:], bv_ap[:])
        bq_t = consts.tile([128, JT], F32)
        nc.scalar.dma_start(bq_t[:], bq_ap[:])
        bk_t = consts.tile([128, JT], F32)
        nc.scalar.dma_start(bk_t[:], bk_ap[:])
        # den staging: partition 0 = even head of pair, partition 64 = odd
        den_all = consts.tile([65, JT * S], F32)

        # ---- weight + x DMAs (issued up front; deps gate consumers) ----
        xT = xT_pool.tile([128, IT, S], BF16)
        for half in range(2):
            for i in range(IT):
                eng = nc.sync if i % 2 == 0 else nc.gpsimd
                eng.dma_start(
                    xT[:, i, half * 1024 : (half + 1) * 1024],
                    xT_ap[i * 128 : (i + 1) * 128, half * 1024 : (half + 1) * 1024],
                )

        wv_tiles = []
        for i in range(IT):
            wv = w_pool.tile([128, JW], BF16, name=f"wv_{i}")
            nc.scalar.dma_start(wv[:], wv_ap[i * 128 : (i + 1) * 128, :])
            wv_tiles.append(wv)
        w_tiles = {}
        for key, w_ap in (("q", wq_ap), ("k", wk_ap)):
            for i in range(IT):
                w = w_pool.tile([128, JW], BF16, name=f"w{key}_{i}")
                nc.scalar.dma_start(w[:], w_ap[i * 128 : (i + 1) * 128, :])
                w_tiles[(key, i)] = w

        # v bias broadcast to all 128 partitions via ones-matmul
        bvb = consts.tile([128, JW], F32)
        bvb_ps = ps_pq.tile([128, 512], F32, tag="pq")
        _pe(nc.tensor.matmul(bvb_ps[:], ones1[:], bvrow[:], start=True, stop=True))
        nc.vector.tensor_copy(bvb[:], bvb_ps[:])

        qT = qkt_pool.tile([128, JT, S], BF16)
        kT = qkt_pool.tile([128, JT, S], BF16)

        # ---- phase A: v projection into vp[tk, head, 64] (per t tile) ----
        vp_tiles = {}

        def emit_pv(t):
            pv = ps_pq.tile([128, 512], F32, tag="pq", name=f"pv_{t}")
            for i in range(IT):
                _pe(nc.tensor.matmul(
                    pv[:],
                    xT[:, i, t * 128 : (t + 1) * 128],
                    wv_tiles[i][:],
                    start=(i == 0),
                    stop=(i == IT - 1),
                ))
            vp = vp_pool.tile([128, HPC, HD], BF16)
            nc.vector.tensor_add(
                vp[:],
                pv[:].rearrange("p (h d) -> p h d", h=HPC),
                bvb[:].rearrange("p (h d) -> p h d", h=HPC),
            )
            vp_tiles[t] = vp

        # ---- phase B: qT/kT projection chains ----
        def emit_proj_chain(key, j, c):
            b_t, dstT = (bq_t, qT) if key == "q" else (bk_t, kT)
            pq = ps_pq.tile([128, 512], F32, tag="pq", name=f"pq_{key}_{j}_{c}")
            for i in range(IT):
                _pe(nc.tensor.matmul(
                    pq[:],
                    w_tiles[(key, i)][:, j * 128 : (j + 1) * 128],
                    xT[:, i, c * 512 : (c + 1) * 512],
                    start=(i == 0),
                    stop=(i == IT - 1),
                ))
            nc.vector.tensor_scalar_add(
                dstT[:, j, c * 512 : (c + 1) * 512], pq[:], b_t[:, j : j + 1]
            )

        # work queue of projection/pv units, consumed just-in-time between
        # attention blocks (all emitted PE work stays back-to-back)
        units = []
        for c in range(1, CH):
            units += [("pv", t) for t in range(4 * c, 4 * c + 4)]
            units += [("q", 0, c), ("k", 0, c)]
        for p in range(1, JT):
            for c in range(CH):
                units += [("q", p, c), ("k", p, c)]
        unit_pos = [0]

        def consume_unit():
            u = units[unit_pos[0]]
            unit_pos[0] += 1
            if u[0] == "pv":
                emit_pv(u[1])
            else:
                emit_proj_chain(u[0], u[1], u[2])

        def drain_units(upto):
            while unit_pos[0] < upto:
                consume_unit()

        def prereq(p, c):
            if p == 0:
                return 6 * c
            return 18 + 8 * (p - 1) + 2 * (c + 1)

        # prologue: minimum for attention (p0, c0)
        for t in range(4):
            emit_pv(t)
        emit_proj_chain("q", 0, 0)
        emit_proj_chain("k", 0, 0)

        # ---- phase C: attention blocks, c-outer j-inner ----
        av_tiles = {}
        dn_tiles = {}
        pts_tiles = {}

        def emit_scores(p, c, j, off):
            N = 512 - off
            tq0 = c * 512 + off
            sc = ps_sc.tile([128, 1024], F32, tag="sc", name=f"sc_{p}_{c}_{j}")
            _pe(nc.tensor.matmul(
                sc[:, 0:N],
                kT[0:64, p, j * 128 : (j + 1) * 128],
                qT[0:64, p, tq0 : tq0 + N],
                start=True,
                stop=True,
                tile_position=(0, 0),
            ))
            _pe(nc.tensor.matmul(
                sc[:, 512 : 512 + N],
                kT[64:128, p, j * 128 : (j + 1) * 128],
                qT[64:128, p, tq0 : tq0 + N],
                start=True,
                stop=True,
                tile_position=(64, 0),
            ))
            return sc

        def emit_tail(p, c, j, off, sc):
            N = 512 - off
            jmax = 4 * c + 3
            if (p, c) not in av_tiles:
                av_tiles[(p, c)] = ps_av.tile(
                    [128, 512], F32, tag="av", name=f"av_{p}_{c}"
                )
                dn_tiles[(p, c)] = ps_dn.tile(
                    [65, 512], F32, tag="dn", name=f"dn_{p}_{c}"
                )
                pts_tiles[(p, c)] = pts_pool.tile(
                    [128, 1024], BF16, tag="pts", name=f"pts_{p}_{c}"
                )
            av = av_tiles[(p, c)]
            dn = dn_tiles[(p, c)]
            pts = pts_tiles[(p, c)]
            if j == 0:
                pt = pts  # exp writes the ptsum accumulator directly
            else:
                pt = pt_pool.tile(
                    [128, 1024], BF16, tag="pt", name=f"pt_{p}_{c}_{j}"
                )
            # single exp covers both heads; [N:512) is stale-but-bounded data
            nc.scalar.activation(pt[:, 0 : 512 + N], sc[:, 0 : 512 + N], ACT_EXP)
            if j >= 4 * c:
                nc.vector.tensor_mul(
                    pt[:, 0:1024].rearrange("p (g q) -> p g q", g=2)[:, :, 0:128],
                    pt[:, 0:1024].rearrange("p (g q) -> p g q", g=2)[:, :, 0:128],
                    tri2[:],
                )
            _pe(nc.tensor.matmul(
                av[0:64, off : off + N],
                vp_tiles[j][:, 2 * p, :],
                pt[:, 0:N],
                start=(j == 0),
                stop=(j == jmax),
                skip_group_check=True,
                tile_position=(0, 0),
            ))
            _pe(nc.tensor.matmul(
                av[64:128, off : off + N],
                vp_tiles[j][:, 2 * p + 1, :],
                pt[:, 512 : 512 + N],
                start=(j == 0),
                stop=(j == jmax),
                skip_group_check=True,
                tile_position=(0, 64),
            ))
            if j > 0:
                if off == 0:
                    nc.vector.tensor_add(
                        pts[:, 0:1024], pts[:, 0:1024], pt[:, 0:1024]
                    )
                else:
                    psrc = pt[:, 0:1024].rearrange("p (g q) -> p g q", g=2)[
                        :, :, 0:N
                    ]
                    pdst = pts[:, 0:1024].rearrange("p (g q) -> p g q", g=2)[
                        :, :, off : off + N
                    ]
                    nc.vector.tensor_add(pdst, pdst, psrc)
            if j == jmax:
                _pe(nc.tensor.matmul(
                    dn[0:1, :],
                    onesb[:],
                    pts[:, 0:512],
                    start=True,
                    stop=True,
                    skip_group_check=True,
                    tile_position=(0, 0),
                ))
                _pe(nc.tensor.matmul(
                    dn[64:65, :],
                    onesb[:],
                    pts[:, 512:1024],
                    start=True,
                    stop=True,
                    skip_group_check=True,
                    tile_position=(0, 64),
                ))
                o = avs_pool.tile([128, 512], BF16, tag="o", name=f"o_{p}_{c}")
                nc.vector.tensor_copy(o[:], av[:])
                nc.sync.dma_start(
                    oav_ap[p, :, c * 512 : (c + 1) * 512], o[:]
                )
                d0 = p * S + c * 512
                nc.vector.tensor_copy(
                    den_all[0:1, d0 : d0 + 512], dn[0:1, :]
                )
                nc.vector.tensor_copy(
                    den_all[64:65, d0 : d0 + 512], dn[64:65, :]
                )
                del av_tiles[(p, c)]
                del dn_tiles[(p, c)]
                del pts_tiles[(p, c)]

        pending = []
        since_unit = [0]
        for p in range(JT):
            for c in range(CH):
                drain_units(prereq(p, c))
                for j in range(4 * c + 4):
                    off = 0 if j < 4 * c else 128 * (j - 4 * c)
                    sc = emit_scores(p, c, j, off)
                    pending.append((p, c, j, off, sc))
                    if len(pending) > 2:
                        emit_tail(*pending.pop(0))
                    since_unit[0] += 1
                    if since_unit[0] >= 2 and unit_pos[0] < len(units):
                        since_unit[0] = 0
                        consume_unit()
        while pending:
            emit_tail(*pending.pop(0))
        drain_units(len(units))
        nc.sync.dma_start(odn_ap[0:1, :], den_all[0:1, :])
        nc.sync.dma_start(odn_ap[1:2, :], den_all[64:65, :])

    _split_sync_waits(nc)
    return nc


_NC_CACHE = {}


def _get_program():
    if "nc" not in _NC_CACHE:
        _NC_CACHE["nc"] = _build_program()
    return _NC_CACHE["nc"]


def _host_prep(inputs):
    scale = 1.0 / math.sqrt(HD)
    import ml_dtypes

    tri = (
        np.arange(128)[None, :] >= np.arange(128)[:, None]
    ).astype(np.float32)
    tri2 = np.ascontiguousarray(
        np.broadcast_to(tri[:, None, :], (128, 2, 128))
    ).astype(ml_dtypes.bfloat16)
    w_eff = {}
    for name in ("q", "k", "v"):
        W = np.asarray(inputs[f"W{name}"], np.float64)
        A = np.asarray(inputs[f"A{name}"], np.float64)
        Bm = np.asarray(inputs[f"B{name}"], np.float64)
        w_eff[name] = W + LORA_SCALING * (A @ Bm)
    xT_b = []
    for b in range(B):
        xb = np.asarray(inputs["hidden_states"], np.float32)[b]
        xT_b.append(np.ascontiguousarray(xb.T).astype(ml_dtypes.bfloat16))
    in_maps = []
    for c in range(N_CORES):
        b, hg = c // 2, c % 2
        sl = slice(hg * JW, (hg + 1) * JW)
        bq = np.asarray(inputs["bq"], np.float64)[sl] * scale
        bk = np.asarray(inputs["bk"], np.float64)[sl]
        bv = np.asarray(inputs["bv"], np.float64)[sl]
        in_maps.append(
            {
                "xT": xT_b[b],
                "wq": np.ascontiguousarray(
                    (w_eff["q"][:, sl] * scale)
                ).astype(ml_dtypes.bfloat16),
                "wk": np.ascontiguousarray(w_eff["k"][:, sl]).astype(ml_dtypes.bfloat16),
                "wv": np.ascontiguousarray(w_eff["v"][:, sl]).astype(ml_dtypes.bfloat16),
                "bq": np.ascontiguousarray(
                    bq.astype(np.float32).reshape(JT, 128).T
                ),
                "bk": np.ascontiguousarray(
                    bk.astype(np.float32).reshape(JT, 128).T
                ),
                "bv": bv.astype(np.float32).reshape(1, JW).astype(ml_dtypes.bfloat16),
                "tri2": tri2,
            }
        )
    return in_maps


def _host_finish(results):
    out = np.empty((B, S, NH * HD), np.float32)
    for c in range(N_CORES):
        b, hg = c // 2, c % 2
        av = results[c]["out_av"].astype(np.float32).reshape(JT, 2, HD, S)
        # out_den [2, JT*S]: row hh, col p*S + tq
        den = results[c]["out_den"].reshape(2, JT, 1, S).transpose(1, 0, 2, 3)
        heads = av / den                      # [p, hh, d, tq]
        heads = heads.transpose(3, 0, 1, 2).reshape(S, JW)
        out[b, :, hg * JW : (hg + 1) * JW] = heads
    return out


def kernel(**inputs) -> np.ndarray:
    in_maps = _host_prep(inputs)
    nc = _get_program()
    res = run_bass_kernel_spmd(nc, in_maps, list(range(N_CORES)))
    return _host_finish(res.results)


if __name__ == "__main__":
    import reference

    inputs = {k: np.asarray(v) for k, v in reference.setup_inputs().items()}
    expected = np.asarray(reference.reference(**inputs))
    actual = kernel(**inputs)
    err = np.abs(actual - expected)
    print("max abs err:", err.max())
    print("scale-relative:", err.max() / np.abs(expected).max())
